# revision 30
# baseline (speedup 1.0000x reference)
# Causal self-attention on 8 NeuronCores (Trainium2, Bass/Tile). v4.
#
# Problem: B=2, T=2048, C=1024, H=16 heads (hd=64).
#   qkv = x @ W_qkv + b_qkv ; per-head causal softmax attention ; y = att_out @ W_proj + b_proj
#
# Sharding: tensor-parallel over heads x data-parallel over batch.
#   core = b*4 + g   (b in {0,1} batch, g in {0..3} head group of 4 heads)
#
# v6 (from v3).  Changes driven by HW NTFF traces:
#   - all matmuls bf16 (was f32r): f32r disables fast-weight-load, so every
#     matmul serialized with its 128-cycle LDWEIGHTS (MATMUL busy 232us +
#     LDW 126us).  bf16 hides the weight load (FWL + background buffer):
#     MATMUL busy -> ~141us.  bf16 error (~0.5% rel) is well inside the 2e-2
#     gate.  fp8 was tried and FAILS (6.4e-2): attention output is an average
#     of uncorrelated v's, so element noise in scores/probs/values passes
#     through at full relative strength.
#   - S matmuls (contraction = hd = 64) head-pair packed: heads (2h, 2h+1)
#     live on partitions 0:64 / 64:128 of qkT, so their S matmuls emit as
#     concurrent 64x128 row tiles (T0/T8) into different PSUM banks -> the
#     K=64 half-array waste disappears.
#   - softmax reciprocal: DVE reciprocal on [1,512] is ~4us (single lane,
#     ~9.4 cyc/elem).  Widen both heads' denominator rows into [128,8] via
#     one DMA, one reciprocal (~200ns), one DMA back, one broadcast for both
#     heads.  63us -> ~2us.
#   - even head's normalize multiply writes AT[0:64] directly (lane-aligned);
#     only the odd head needs the partition-moving DMA.
#   - proj evacuation all on DVE (ACT is busy with exp during B2).
#   - x / W_qkv / W_v / W_proj shipped as bf16: halves the phase-A DMA.

import numpy as np
import ml_dtypes

T = 2048
C = 1024
HL = 4          # heads per core
HD = 64
CL = HL * HD    # 256 local channels
P = 128

BF16 = ml_dtypes.bfloat16
FP8 = ml_dtypes.float8_e4m3

_cache = {}


def _build_nc():
    import concourse.bass as bass
    import concourse.mybir as mybir
    import concourse.tile as tile
    from concourse import bacc
    from contextlib import ExitStack

    f32 = mybir.dt.float32
    bf16 = mybir.dt.bfloat16
    f8 = mybir.dt.float8e4
    DR = mybir.MatmulPerfMode.DoubleRow
    EXP = mybir.ActivationFunctionType.Exp
    IDENT = mybir.ActivationFunctionType.Identity

    nc = bacc.Bacc("TRN2", target_bir_lowering=False)
    xT_d = nc.declare_dram_parameter("xT", [C, T], bf16, isOutput=False)
    wqk_d = nc.declare_dram_parameter("wqk", [C, 2 * CL], bf16, isOutput=False)
    wv_d = nc.declare_dram_parameter("wv", [C, CL], bf16, isOutput=False)
    bqk_d = nc.declare_dram_parameter("bqk", [P, 4], f32, isOutput=False)
    bv_d = nc.declare_dram_parameter("bv", [1, CL], f32, isOutput=False)
    kbias_d = nc.declare_dram_parameter("kbias", [P, 16], f32, isOutput=False)
    wproj_d = nc.declare_dram_parameter("wproj", [CL, C], bf16, isOutput=False)
    yT_d = nc.declare_dram_parameter("yT", [C, T], bf16, isOutput=True)

    NT = T // P       # 16 t-tiles of 128
    NCC = C // P      # 8 contraction chunks of 128
    NQ = T // 512     # 4 q-quads of 512

    with tile.TileContext(nc) as tc, ExitStack() as ctx:
        singles = ctx.enter_context(tc.tile_pool(name="singles", bufs=1))

        # persistent SBUF
        qkT = singles.tile([P, 4, T], bf16)        # rows: [q f0,q f1,k f0,k f1]
        vv = singles.tile([P, NT, HL, HD + 1], bf16)  # v + ones col per head
        AT = singles.tile([P, 2, T], bf16)         # attention out^T (c' x t)
        tri01 = singles.tile([P, P], bf16)         # lower-tri 1.0 / 0.0
        kbias_sb = singles.tile([P, 16], f32)
        bqk_sb = singles.tile([P, 4], f32)
        bv_sb = singles.tile([P, HL, HD], f32)
        wproj_sb = singles.tile([P, 2, C], bf16)
        xc = singles.tile([P, NCC, T], bf16)
        wqkc = singles.tile([P, NCC, 2 * CL], bf16)
        wv_sb = singles.tile([P, NCC, CL], bf16)

        # weights/params on the gpsimd (SWDGE) queue, xT chunks on the sync
        # (HWDGE) queue.  The first q/k matmul needs only wqkc[:,0,:] +
        # xc[:,0,:], so those are issued FIRST on their queues; tri01 setup
        # (gpsimd engine work, needed only in B1) comes after the DMA issues.
        for ci in range(NCC):
            nc.sync.dma_start(out=xc[:, ci, :], in_=xT_d[ci * P:(ci + 1) * P, :])
        for ci in range(2):
            nc.gpsimd.dma_start(
                out=wqkc[:, ci, :], in_=wqk_d[ci * P:(ci + 1) * P, :]
            )
        nc.gpsimd.dma_start(out=kbias_sb, in_=kbias_d[:])
        nc.gpsimd.dma_start(out=bqk_sb, in_=bqk_d[:])
        nc.gpsimd.dma_start(
            out=bv_sb,
            in_=bv_d[:].rearrange("o (h d) -> o h d", h=HL).to_broadcast([P, HL, HD]),
        )
        for ci in range(2, NCC):
            nc.gpsimd.dma_start(
                out=wqkc[:, ci, :], in_=wqk_d[ci * P:(ci + 1) * P, :]
            )
        nc.gpsimd.dma_start(out=wv_sb, in_=wv_d[:].rearrange("(o p) n -> p o n", p=P))
        nc.gpsimd.dma_start(out=wproj_sb, in_=wproj_d[:].rearrange("(o p) n -> p o n", p=P))
        nc.vector.memset(vv[:, :, :, HD], 1.0)

        # tri01[k, q] = 1.0 where q >= k else 0.0
        nc.gpsimd.memset(tri01, 1.0)
        nc.gpsimd.affine_select(
            out=tri01,
            in_=tri01,
            compare_op=mybir.AluOpType.is_ge,
            fill=0.0,
            base=0,
            pattern=[[1, P]],
            channel_multiplier=-1,
        )

        with (
            tc.tile_pool(name="ptp", bufs=4) as ptp,
            tc.tile_pool(name="ep", bufs=3) as ep,
            tc.tile_pool(name="yst", bufs=3) as yst,
        ):
            phA_cm = tc.tile_pool(name="phA", bufs=1, space="PSUM")
            phA = phA_cm.__enter__()
            # ---- q/k group A (fi 0,2 = q,k of heads 0/1): ci-outer ----
            groupA = [(fi, tj) for fi in (0, 2) for tj in range(4)]
            pqA = [
                phA.tile([P, 512], f32, name=f"pqA{gi}")
                for gi in range(len(groupA))
            ]
            for ci in range(NCC):
                for gi, (fi, tj) in enumerate(groupA):
                    nc.tensor.matmul(
                        pqA[gi],
                        lhsT=wqkc[:, ci, fi * P:(fi + 1) * P],
                        rhs=xc[:, ci, tj * 512:(tj + 1) * 512],
                        start=(ci == 0),
                        stop=(ci == NCC - 1),
                        skip_group_check=True,
                    )
            for gi, (fi, tj) in enumerate(groupA):
                # bias-add evacuation on the (idle in phase A) ACT engine
                nc.scalar.activation(
                    out=qkT[:, fi, tj * 512:(tj + 1) * 512],
                    in_=pqA[gi],
                    func=IDENT,
                    bias=bqk_sb[:, fi:fi + 1],
                )

            # ---- v pass: 16 half-bank accum groups reusing the 8 banks ----
            for ti in range(NT):
                pv = pqA[ti % 8][:, (ti // 8) * CL:(ti // 8 + 1) * CL]
                for ci in range(NCC):
                    nc.tensor.matmul(
                        pv,
                        lhsT=xc[:, ci, ti * P:(ti + 1) * P],
                        rhs=wv_sb[:, ci, :],
                        start=(ci == 0),
                        stop=(ci == NCC - 1),
                        skip_group_check=True,
                    )
                nc.vector.tensor_add(
                    out=vv[:, ti, :, 0:HD],
                    in0=pv.rearrange("p (h d) -> p h d", h=HL),
                    in1=bv_sb,
                )
                # key-padding mask: zero this key's v row AND its ones-col
                nc.vector.tensor_scalar_mul(
                    out=vv[:, ti, :, :],
                    in0=vv[:, ti, :, :],
                    scalar1=kbias_sb[:, ti:ti + 1],
                )

            phA_cm.__exit__(None, None, None)

            groupB = [(fi, tj) for fi in (1, 3) for tj in range(4)]

            def qk_groupB(k):
                fi, tj = groupB[k]
                pq = ps_y.tile([P, 512], f32, name=f"pqB", tag="y")
                for ci in range(NCC):
                    nc.tensor.matmul(
                        pq,
                        lhsT=wqkc[:, ci, fi * P:(fi + 1) * P],
                        rhs=xc[:, ci, tj * 512:(tj + 1) * 512],
                        start=(ci == 0),
                        stop=(ci == NCC - 1),
                    )
                nc.vector.tensor_scalar_add(
                    out=qkT[:, fi, tj * 512:(tj + 1) * 512],
                    in0=pq,
                    scalar1=bqk_sb[:, fi:fi + 1],
                )

            def attention_pair(qq, hp, po0, po1):
                # heads (2*hp, 2*hp+1) processed together.  Their q/k rows sit
                # on SBUF partitions 0:64 / 64:128 of qkT, so the two S
                # matmuls of a chunk are emitted back-to-back as 64x128 row
                # tiles (T0 / T8) writing different PSUM banks -> they run
                # CONCURRENTLY on the PE array (2x on the K=64 S matmuls).
                qs = qq * 512
                fo = hp
                qT = qkT[:, fo, :]
                kT = qkT[:, 2 + fo, :]
                pos = (po0, po1)
                # full (below-diagonal) chunks, in groups of 2 to keep the
                # 64-row-mode <-> 128-row-mode switches as rare as v4
                for jp in range(2 * qq):
                    j0 = 2 * jp
                    pss = [
                        ps_s.tile([P, 2, 512], f32, name="ps2", tag="s")
                        for _ in range(2)
                    ]
                    pTs = [
                        ptp.tile([P, 2, 512], bf16, name="pT2", tag="p")
                        for _ in range(2)
                    ]
                    for m in range(2):
                        for i in range(2):
                            b = i * HD
                            nc.tensor.matmul(
                                pss[m][:, i, :],
                                lhsT=kT[b:b + HD, (j0 + m) * P:(j0 + m + 1) * P],
                                rhs=qT[b:b + HD, qs:qs + 512],
                                start=True,
                                stop=True,
                            )
                    for m in range(2):
                        nc.scalar.activation(
                            out=pTs[m], in_=pss[m], func=EXP, scale=0.125,
                        )
                    for m in range(2):
                        for i in range(2):
                            nc.tensor.matmul(
                                pos[i],
                                lhsT=vv[:, j0 + m, 2 * hp + i, :],
                                rhs=pTs[m][:, i, :],
                                start=(j0 + m == 0),
                                stop=False,
                            )
                # diagonal-region chunks (o = 0..3), width-trimmed to >=256
                for o in range(4):
                    j = 4 * qq + o
                    d0 = 128 * o
                    a = min(d0, 256)
                    ps = ps_s.tile([P, 2, 512], f32, name="psd", tag="s")
                    pT = ptp.tile([P, 2, 512], bf16, name="pTd", tag="p")
                    for i in range(2):
                        b = i * HD
                        nc.tensor.matmul(
                            ps[:, i, a:],
                            lhsT=kT[b:b + HD, j * P:(j + 1) * P],
                            rhs=qT[b:b + HD, qs + a:qs + 512],
                            start=True,
                            stop=True,
                        )
                    nc.scalar.activation(
                        out=pT[:, :, d0:], in_=ps[:, :, d0:],
                        func=EXP, scale=0.125,
                    )
                    for i in range(2):
                        nc.vector.tensor_mul(
                            out=pT[:, i, d0:d0 + P],
                            in0=pT[:, i, d0:d0 + P],
                            in1=tri01,
                        )
                        if d0 > a:
                            # o==3: cols [256,384) are causally invalid
                            nc.vector.memset(pT[:, i, a:d0], 0.0)
                    for i in range(2):
                        nc.tensor.matmul(
                            pos[i][:, a:],
                            lhsT=vv[:, j, 2 * hp + i, :],
                            rhs=pT[:, i, a:],
                            start=(j == 0),
                            stop=(o == 3),
                        )

            def normalize_pair(qq, hp, po0, po1, ob2):
                # denominators of heads (2*hp, 2*hp+1) sit on partition 64 of
                # po0/po1.  Widen [1024] -> [128,8] in ONE DMA so the
                # reciprocal runs 8 elem/lane instead of 512 on one lane;
                # scatter back in one DMA; one broadcast for both heads.
                # The two row-copies run in parallel on ACT and DVE - this
                # chain's latency gates the proj filler each iteration.
                qs = qq * 512
                # Evacuate BOTH accumulators to SBUF right away (full-tile
                # copies, ACT + DVE in parallel) so the po PSUM banks free up
                # at the START of this chain, not after the normalize muls -
                # otherwise the next quad's AV matmuls stall ~5us on the
                # ps_o pool.  Row 64 is the ones-column denominator.
                st = ep.tile([HD + 1, 2, 512], f32, name="st", tag="st")
                nc.scalar.activation(out=st[:, 0, :], in_=po0, func=IDENT)
                nc.vector.tensor_copy(out=st[:, 1, :], in_=po1)
                dwide = ep.tile([P, 8], f32, name="dwide", tag="dw")
                nc.sync.dma_start(
                    out=dwide,
                    in_=st[HD:HD + 1, :, :].rearrange(
                        "o h (p c) -> o (h p) c", p=P // 2
                    ),
                )
                rwide = ep.tile([P, 8], f32, name="rwide", tag="rw")
                with nc.allow_low_precision(
                    reason="reciprocal of softmax denom; 2^-11 rel"
                ):
                    nc.vector.reciprocal(out=rwide, in_=dwide)
                den2 = ep.tile([1, 1024], f32, name="den2", tag="den")
                nc.sync.dma_start(
                    out=den2.rearrange("o (p c) -> o p c", p=P),
                    in_=rwide,
                )
                rb = ep.tile([HD, 1024], f32, name="rb", tag="rb")
                nc.gpsimd.partition_broadcast(rb, den2)
                # even head (partitions 0:64 of AT): the normalize multiply
                # writes AT directly - no partition-moving DMA needed
                nc.vector.tensor_mul(
                    out=AT[0:HD, hp, qs:qs + 512], in0=st[0:HD, 0, :],
                    in1=rb[:, 0:512],
                )
                # odd head: DVE is lane-aligned, so route via ob2 + DMA to
                # reach AT partitions 64:128
                nc.vector.tensor_mul(
                    out=ob2[:, 1, :], in0=st[0:HD, 1, :],
                    in1=rb[:, 512:1024],
                )
                nc.gpsimd.dma_start(
                    out=AT[HD:P, hp, qs:qs + 512],
                    in_=ob2[:, 1, :],
                )

            def proj(qq):
                for co in range(C // P):
                    py = ps_y.tile([P, 512], f32, name="py", tag="y")
                    for cc in range(2):
                        nc.tensor.matmul(
                            py,
                            lhsT=wproj_sb[:, cc, co * P:(co + 1) * P],
                            rhs=AT[:, cc, qq * 512:(qq + 1) * 512],
                            start=(cc == 0),
                            stop=(cc == 1),
                        )
                    yt = yst.tile([P, 512], bf16, name="yt", tag="yt")
                    nc.vector.tensor_copy(out=yt, in_=py)
                    dma_eng = nc.sync if co % 2 == 0 else nc.gpsimd
                    dma_eng.dma_start(
                        out=yT_d[co * P:(co + 1) * P, qq * 512:(qq + 1) * 512],
                        in_=yt,
                    )

            with (
                tc.tile_pool(name="ps_s", bufs=2, space="PSUM") as ps_s,
                tc.tile_pool(name="ps_o", bufs=2, space="PSUM") as ps_o,
                tc.tile_pool(name="ps_y", bufs=2, space="PSUM") as ps_y,
            ):
                # ---- B: single loop, head pairs interleaved per quad.
                # Each pair's normalize chain overlaps the other pair's
                # attention / fillers.  qk_groupB(qq) + qk_groupB(4+qq)
                # produce q (f1, quad qq) and k (f3, chunk range tj=qq) of
                # heads 2/3 just-in-time: attention_pair(qq, 1) needs k
                # columns 0:(qq+1)*512, i.e. tj <= qq - all emitted by then.
                for qq in range(NQ):
                    ob01 = ep.tile([HD, 2, 512], bf16, name="ob01", tag="ob2")
                    po0 = ps_o.tile([HD + 1, 512], f32, name="po", tag="o")
                    po1 = ps_o.tile([HD + 1, 512], f32, name="po", tag="o")
                    attention_pair(qq, 0, po0, po1)
                    qk_groupB(qq)
                    qk_groupB(4 + qq)
                    normalize_pair(qq, 0, po0, po1, ob01)
                    ob23 = ep.tile([HD, 2, 512], bf16, name="ob23", tag="ob2")
                    po0b = ps_o.tile([HD + 1, 512], f32, name="po", tag="o")
                    po1b = ps_o.tile([HD + 1, 512], f32, name="po", tag="o")
                    attention_pair(qq, 1, po0b, po1b)
                    if qq > 0:
                        proj(qq - 1)
                    normalize_pair(qq, 1, po0b, po1b, ob23)
                proj(NQ - 1)

    return nc


def _get_nc():
    if "nc" not in _cache:
        nc = _build_nc()
        nc.finalize()  # runs the Bacc pass pipeline (reg alloc, wait splitting)
        _cache["nc"] = nc
    return _cache["nc"]


def _make_in_maps(x, attn_mask, W_qkv, b_qkv, W_proj):
    x = np.asarray(x, dtype=np.float32)
    attn_mask = np.asarray(attn_mask)
    W_qkv = np.asarray(W_qkv, dtype=np.float32)
    b_qkv = np.asarray(b_qkv, dtype=np.float32)
    W_proj = np.asarray(W_proj, dtype=np.float32)

    xT = [np.ascontiguousarray(x[b].T.astype(BF16)) for b in range(2)]

    in_maps = []
    for core in range(8):
        b, g = core // 4, core % 4
        s = slice(CL * g, CL * (g + 1))
        wq = W_qkv[:, 0 * C:1 * C][:, s]
        wk = W_qkv[:, 1 * C:2 * C][:, s]
        wv = W_qkv[:, 2 * C:3 * C][:, s]
        bq = b_qkv[0 * C:1 * C][s]
        bk = b_qkv[1 * C:2 * C][s]
        bv = b_qkv[2 * C:3 * C][s]
        bqk = np.concatenate([bq, bk]).reshape(4, P).T  # [128,4], f = fi*128+p
        kbias = (attn_mask[b] != 0).astype(np.float32)  # 0/1 key mask
        in_maps.append({
            "xT": xT[b],
            "wqk": np.ascontiguousarray(
                np.concatenate([wq, wk], axis=1).astype(BF16)),
            "wv": np.ascontiguousarray(wv.astype(BF16)),
            "bqk": np.ascontiguousarray(bqk),
            "bv": np.ascontiguousarray(bv.reshape(1, CL)),
            "kbias": np.ascontiguousarray(kbias.reshape(16, P).T),
            "wproj": np.ascontiguousarray(W_proj[s, :].astype(BF16)),
        })
    return in_maps


def kernel(x, attn_mask, W_qkv, b_qkv, W_proj, b_proj, _trace=False):
    from concourse.bass_utils import run_bass_kernel_spmd

    nc = _get_nc()
    in_maps = _make_in_maps(x, attn_mask, W_qkv, b_qkv, W_proj)
    res = run_bass_kernel_spmd(nc, in_maps, list(range(8)), trace=_trace)
    outs = res.results

    b_proj = np.asarray(b_proj, dtype=np.float32)
    y = np.empty((2, T, C), dtype=np.float32)
    for b in range(2):
        acc = outs[b * 4]["yT"].T.astype(np.float32)
        for g in range(1, 4):
            acc = acc + outs[b * 4 + g]["yT"].T.astype(np.float32)
        y[b] = acc + b_proj
    if _trace:
        return y, res
    return y


# revision 33
# speedup vs baseline: 1.1501x; 1.1501x over previous
# Causal self-attention on 8 NeuronCores (Trainium2, Bass/Tile). v4.
#
# Problem: B=2, T=2048, C=1024, H=16 heads (hd=64).
#   qkv = x @ W_qkv + b_qkv ; per-head causal softmax attention ; y = att_out @ W_proj + b_proj
#
# Sharding: tensor-parallel over heads x data-parallel over batch.
#   core = b*4 + g   (b in {0,1} batch, g in {0..3} head group of 4 heads)
#
# v6 (from v3).  Changes driven by HW NTFF traces:
#   - all matmuls bf16 (was f32r): f32r disables fast-weight-load, so every
#     matmul serialized with its 128-cycle LDWEIGHTS (MATMUL busy 232us +
#     LDW 126us).  bf16 hides the weight load (FWL + background buffer):
#     MATMUL busy -> ~141us.  bf16 error (~0.5% rel) is well inside the 2e-2
#     gate.  fp8 was tried and FAILS (6.4e-2): attention output is an average
#     of uncorrelated v's, so element noise in scores/probs/values passes
#     through at full relative strength.
#   - S matmuls (contraction = hd = 64) head-pair packed: heads (2h, 2h+1)
#     live on partitions 0:64 / 64:128 of qkT, so their S matmuls emit as
#     concurrent 64x128 row tiles (T0/T8) into different PSUM banks -> the
#     K=64 half-array waste disappears.
#   - softmax reciprocal: DVE reciprocal on [1,512] is ~4us (single lane,
#     ~9.4 cyc/elem).  Widen both heads' denominator rows into [128,8] via
#     one DMA, one reciprocal (~200ns), one DMA back, one broadcast for both
#     heads.  63us -> ~2us.
#   - even head's normalize multiply writes AT[0:64] directly (lane-aligned);
#     only the odd head needs the partition-moving DMA.
#   - proj evacuation all on DVE (ACT is busy with exp during B2).
#   - x / W_qkv / W_v / W_proj shipped as bf16: halves the phase-A DMA.

import numpy as np
import ml_dtypes

T = 2048
C = 1024
HL = 4          # heads per core
HD = 64
CL = HL * HD    # 256 local channels
P = 128

BF16 = ml_dtypes.bfloat16
FP8 = ml_dtypes.float8_e4m3

_cache = {}


def _build_nc():
    import concourse.bass as bass
    import concourse.mybir as mybir
    import concourse.tile as tile
    from concourse import bacc
    from contextlib import ExitStack

    f32 = mybir.dt.float32
    bf16 = mybir.dt.bfloat16
    f8 = mybir.dt.float8e4
    DR = mybir.MatmulPerfMode.DoubleRow
    EXP = mybir.ActivationFunctionType.Exp
    IDENT = mybir.ActivationFunctionType.Identity

    nc = bacc.Bacc("TRN2", target_bir_lowering=False)
    xT_d = nc.declare_dram_parameter("xT", [C, T], bf16, isOutput=False)
    wqk_d = nc.declare_dram_parameter("wqk", [C, 2 * CL], bf16, isOutput=False)
    wv_d = nc.declare_dram_parameter("wv", [C, CL], bf16, isOutput=False)
    bqk_d = nc.declare_dram_parameter("bqk", [P, 4], f32, isOutput=False)
    bv_d = nc.declare_dram_parameter("bv", [1, CL], f32, isOutput=False)
    kbias_d = nc.declare_dram_parameter("kbias", [P, 16], f32, isOutput=False)
    wproj_d = nc.declare_dram_parameter("wproj", [CL, C], bf16, isOutput=False)
    yT_d = nc.declare_dram_parameter("yT", [C, T], bf16, isOutput=True)

    NT = T // P       # 16 t-tiles of 128
    NCC = C // P      # 8 contraction chunks of 128
    NQ = T // 512     # 4 q-quads of 512

    with tile.TileContext(nc) as tc, ExitStack() as ctx:
        singles = ctx.enter_context(tc.tile_pool(name="singles", bufs=1))

        # persistent SBUF
        qkT = singles.tile([P, 4, T], bf16)        # rows: [q f0,q f1,k f0,k f1]
        vv = singles.tile([P, NT, HL, HD + 1], bf16)  # v + ones col per head
        AT = singles.tile([P, 2, T], bf16)         # attention out^T (c' x t)
        tri01 = singles.tile([P, P], bf16)         # lower-tri 1.0 / 0.0
        kbias_sb = singles.tile([P, 16], f32)
        bqk_sb = singles.tile([P, 4], f32)
        bv_sb = singles.tile([P, HL, HD], f32)
        wproj_sb = singles.tile([P, 2, C], bf16)
        xc = singles.tile([P, NCC, T], bf16)
        wqkc = singles.tile([P, NCC, 2 * CL], bf16)
        wv_sb = singles.tile([P, NCC, CL], bf16)

        # weights/params on the gpsimd (SWDGE) queue, xT chunks on the sync
        # (HWDGE) queue.  The first q/k matmul needs only wqkc[:,0,:] +
        # xc[:,0,:], so those are issued FIRST on their queues; tri01 setup
        # (gpsimd engine work, needed only in B1) comes after the DMA issues.
        for ci in range(NCC):
            nc.sync.dma_start(out=xc[:, ci, :], in_=xT_d[ci * P:(ci + 1) * P, :])
        for ci in range(2):
            nc.gpsimd.dma_start(
                out=wqkc[:, ci, :], in_=wqk_d[ci * P:(ci + 1) * P, :]
            )
        nc.gpsimd.dma_start(out=kbias_sb, in_=kbias_d[:])
        nc.gpsimd.dma_start(out=bqk_sb, in_=bqk_d[:])
        nc.gpsimd.dma_start(
            out=bv_sb,
            in_=bv_d[:].rearrange("o (h d) -> o h d", h=HL).to_broadcast([P, HL, HD]),
        )
        for ci in range(2, NCC):
            nc.gpsimd.dma_start(
                out=wqkc[:, ci, :], in_=wqk_d[ci * P:(ci + 1) * P, :]
            )
        nc.gpsimd.dma_start(out=wv_sb, in_=wv_d[:].rearrange("(o p) n -> p o n", p=P))
        nc.gpsimd.dma_start(out=wproj_sb, in_=wproj_d[:].rearrange("(o p) n -> p o n", p=P))
        nc.vector.memset(vv[:, :, :, HD], 1.0)

        # tri01[k, q] = 1.0 where q >= k else 0.0
        nc.gpsimd.memset(tri01, 1.0)
        nc.gpsimd.affine_select(
            out=tri01,
            in_=tri01,
            compare_op=mybir.AluOpType.is_ge,
            fill=0.0,
            base=0,
            pattern=[[1, P]],
            channel_multiplier=-1,
        )

        with (
            tc.tile_pool(name="ptp", bufs=4) as ptp,
            tc.tile_pool(name="ep", bufs=3) as ep,
            tc.tile_pool(name="yst", bufs=3) as yst,
        ):
            phA_cm = tc.tile_pool(name="phA", bufs=1, space="PSUM")
            phA = phA_cm.__enter__()
            # ---- q/k group A (fi 0,2 = q,k of heads 0/1): ci-outer ----
            groupA = [(fi, tj) for fi in (0, 2) for tj in range(4)]
            pqA = [
                phA.tile([P, 512], f32, name=f"pqA{gi}")
                for gi in range(len(groupA))
            ]
            for ci in range(NCC):
                for gi, (fi, tj) in enumerate(groupA):
                    nc.tensor.matmul(
                        pqA[gi],
                        lhsT=wqkc[:, ci, fi * P:(fi + 1) * P],
                        rhs=xc[:, ci, tj * 512:(tj + 1) * 512],
                        start=(ci == 0),
                        stop=(ci == NCC - 1),
                        skip_group_check=True,
                    )
            for gi, (fi, tj) in enumerate(groupA):
                # bias-add evacuation on the (idle in phase A) ACT engine
                nc.scalar.activation(
                    out=qkT[:, fi, tj * 512:(tj + 1) * 512],
                    in_=pqA[gi],
                    func=IDENT,
                    bias=bqk_sb[:, fi:fi + 1],
                )

            # ---- v pass: 16 half-bank accum groups reusing the 8 banks ----
            for ti in range(NT):
                pv = pqA[ti % 8][:, (ti // 8) * CL:(ti // 8 + 1) * CL]
                for ci in range(NCC):
                    nc.tensor.matmul(
                        pv,
                        lhsT=xc[:, ci, ti * P:(ti + 1) * P],
                        rhs=wv_sb[:, ci, :],
                        start=(ci == 0),
                        stop=(ci == NCC - 1),
                        skip_group_check=True,
                    )
                nc.vector.tensor_add(
                    out=vv[:, ti, :, 0:HD],
                    in0=pv.rearrange("p (h d) -> p h d", h=HL),
                    in1=bv_sb,
                )
                # key-padding mask: zero this key's v row AND its ones-col
                nc.vector.tensor_scalar_mul(
                    out=vv[:, ti, :, :],
                    in0=vv[:, ti, :, :],
                    scalar1=kbias_sb[:, ti:ti + 1],
                )

            phA_cm.__exit__(None, None, None)

            groupB = [(fi, tj) for fi in (1, 3) for tj in range(4)]

            def qk_groupB(k):
                fi, tj = groupB[k]
                pq = ps_y.tile([P, 512], f32, name=f"pqB", tag="y")
                for ci in range(NCC):
                    nc.tensor.matmul(
                        pq,
                        lhsT=wqkc[:, ci, fi * P:(fi + 1) * P],
                        rhs=xc[:, ci, tj * 512:(tj + 1) * 512],
                        start=(ci == 0),
                        stop=(ci == NCC - 1),
                    )
                nc.vector.tensor_scalar_add(
                    out=qkT[:, fi, tj * 512:(tj + 1) * 512],
                    in0=pq,
                    scalar1=bqk_sb[:, fi:fi + 1],
                )

            def attention_pair(qq, hp, po0, po1):
                # heads (2*hp, 2*hp+1) processed together.  Their q/k rows sit
                # on SBUF partitions 0:64 / 64:128 of qkT, so the two S
                # matmuls of a chunk are emitted back-to-back as 64x128 row
                # tiles (T0 / T8) writing different PSUM banks -> they run
                # CONCURRENTLY on the PE array (2x on the K=64 S matmuls).
                qs = qq * 512
                fo = hp
                qT = qkT[:, fo, :]
                kT = qkT[:, 2 + fo, :]
                pos = (po0, po1)
                # full (below-diagonal) chunks, in groups of 2 to keep the
                # 64-row-mode <-> 128-row-mode switches as rare as v4
                for jp in range(2 * qq):
                    j0 = 2 * jp
                    pss = [
                        ps_s.tile([P, 2, 512], f32, name="ps2", tag="s")
                        for _ in range(2)
                    ]
                    pTs = [
                        ptp.tile([P, 2, 512], bf16, name="pT2", tag="p")
                        for _ in range(2)
                    ]
                    for m in range(2):
                        for i in range(2):
                            b = i * HD
                            nc.tensor.matmul(
                                pss[m][:, i, :],
                                lhsT=kT[b:b + HD, (j0 + m) * P:(j0 + m + 1) * P],
                                rhs=qT[b:b + HD, qs:qs + 512],
                                start=True,
                                stop=True,
                            )
                    for m in range(2):
                        nc.scalar.activation(
                            out=pTs[m], in_=pss[m], func=EXP, scale=0.125,
                        )
                    for m in range(2):
                        for i in range(2):
                            nc.tensor.matmul(
                                pos[i],
                                lhsT=vv[:, j0 + m, 2 * hp + i, :],
                                rhs=pTs[m][:, i, :],
                                start=(j0 + m == 0),
                                stop=False,
                            )
                # diagonal-region chunks (o = 0..3), width-trimmed to >=256
                for o in range(4):
                    j = 4 * qq + o
                    d0 = 128 * o
                    a = min(d0, 256)
                    ps = ps_s.tile([P, 2, 512], f32, name="psd", tag="s")
                    pT = ptp.tile([P, 2, 512], bf16, name="pTd", tag="p")
                    for i in range(2):
                        b = i * HD
                        nc.tensor.matmul(
                            ps[:, i, a:],
                            lhsT=kT[b:b + HD, j * P:(j + 1) * P],
                            rhs=qT[b:b + HD, qs + a:qs + 512],
                            start=True,
                            stop=True,
                        )
                    nc.scalar.activation(
                        out=pT[:, :, d0:], in_=ps[:, :, d0:],
                        func=EXP, scale=0.125,
                    )
                    for i in range(2):
                        nc.vector.tensor_mul(
                            out=pT[:, i, d0:d0 + P],
                            in0=pT[:, i, d0:d0 + P],
                            in1=tri01,
                        )
                        if d0 > a:
                            # o==3: cols [256,384) are causally invalid
                            nc.vector.memset(pT[:, i, a:d0], 0.0)
                    for i in range(2):
                        nc.tensor.matmul(
                            pos[i][:, a:],
                            lhsT=vv[:, j, 2 * hp + i, :],
                            rhs=pT[:, i, a:],
                            start=(j == 0),
                            stop=(o == 3),
                        )

            def normalize_pair(qq, hp, po0, po1, ob2):
                # denominators of heads (2*hp, 2*hp+1) sit on partition 64 of
                # po0/po1.  Widen [1024] -> [128,8] in ONE DMA so the
                # reciprocal runs 8 elem/lane instead of 512 on one lane;
                # scatter back in one DMA; one broadcast for both heads.
                # The two row-copies run in parallel on ACT and DVE - this
                # chain's latency gates the proj filler each iteration.
                qs = qq * 512
                # Evacuate BOTH accumulators to SBUF right away (full-tile
                # copies, ACT + DVE in parallel) so the po PSUM banks free up
                # at the START of this chain, not after the normalize muls -
                # otherwise the next quad's AV matmuls stall ~5us on the
                # ps_o pool.  Row 64 is the ones-column denominator.
                st = ep.tile([HD + 1, 2, 512], f32, name="st", tag="st")
                nc.vector.tensor_copy(out=st[:, 0, :], in_=po0)
                nc.vector.tensor_copy(out=st[:, 1, :], in_=po1)
                dwide = ep.tile([P, 8], f32, name="dwide", tag="dw")
                nc.sync.dma_start(
                    out=dwide,
                    in_=st[HD:HD + 1, :, :].rearrange(
                        "o h (p c) -> o (h p) c", p=P // 2
                    ),
                )
                rwide = ep.tile([P, 8], f32, name="rwide", tag="rw")
                with nc.allow_low_precision(
                    reason="reciprocal of softmax denom; 2^-11 rel"
                ):
                    nc.vector.reciprocal(out=rwide, in_=dwide)
                den2 = ep.tile([1, 1024], f32, name="den2", tag="den")
                nc.sync.dma_start(
                    out=den2.rearrange("o (p c) -> o p c", p=P),
                    in_=rwide,
                )
                rb = ep.tile([HD, 1024], f32, name="rb", tag="rb")
                nc.gpsimd.partition_broadcast(rb, den2)
                # even head (partitions 0:64 of AT): the normalize multiply
                # writes AT directly - no partition-moving DMA needed
                nc.vector.tensor_mul(
                    out=AT[0:HD, hp, qs:qs + 512], in0=st[0:HD, 0, :],
                    in1=rb[:, 0:512],
                )
                # odd head: DVE is lane-aligned, so route via ob2 + DMA to
                # reach AT partitions 64:128
                nc.vector.tensor_mul(
                    out=ob2[:, 1, :], in0=st[0:HD, 1, :],
                    in1=rb[:, 512:1024],
                )
                nc.gpsimd.dma_start(
                    out=AT[HD:P, hp, qs:qs + 512],
                    in_=ob2[:, 1, :],
                )

            def proj(qq, tail=False):
                for co in range(C // P):
                    py = ps_y.tile([P, 512], f32, name="py", tag="y")
                    for cc in range(2):
                        nc.tensor.matmul(
                            py,
                            lhsT=wproj_sb[:, cc, co * P:(co + 1) * P],
                            rhs=AT[:, cc, qq * 512:(qq + 1) * 512],
                            start=(cc == 0),
                            stop=(cc == 1),
                        )
                    yt = yst.tile([P, 512], bf16, name="yt", tag="yt")
                    if tail and co % 2 == 0:
                        # final quad: ACT is idle at the tail - split the
                        # evacuation so it doesn't serialize on DVE
                        nc.scalar.copy(out=yt, in_=py)
                    else:
                        nc.vector.tensor_copy(out=yt, in_=py)
                    dma_eng = nc.sync if co % 2 == 0 else nc.gpsimd
                    dma_eng.dma_start(
                        out=yT_d[co * P:(co + 1) * P, qq * 512:(qq + 1) * 512],
                        in_=yt,
                    )

            with (
                tc.tile_pool(name="ps_s", bufs=2, space="PSUM") as ps_s,
                tc.tile_pool(name="ps_o", bufs=2, space="PSUM") as ps_o,
                tc.tile_pool(name="ps_y", bufs=2, space="PSUM") as ps_y,
            ):
                # ---- B: single loop, head pairs interleaved per quad.
                # Each pair's normalize chain overlaps the other pair's
                # attention / fillers.  qk_groupB(qq) + qk_groupB(4+qq)
                # produce q (f1, quad qq) and k (f3, chunk range tj=qq) of
                # heads 2/3 just-in-time: attention_pair(qq, 1) needs k
                # columns 0:(qq+1)*512, i.e. tj <= qq - all emitted by then.
                for qq in range(NQ):
                    ob01 = ep.tile([HD, 2, 512], bf16, name="ob01", tag="ob2")
                    po0 = ps_o.tile([HD + 1, 512], f32, name="po", tag="o")
                    po1 = ps_o.tile([HD + 1, 512], f32, name="po", tag="o")
                    attention_pair(qq, 0, po0, po1)
                    qk_groupB(qq)
                    qk_groupB(4 + qq)
                    normalize_pair(qq, 0, po0, po1, ob01)
                    ob23 = ep.tile([HD, 2, 512], bf16, name="ob23", tag="ob2")
                    po0b = ps_o.tile([HD + 1, 512], f32, name="po", tag="o")
                    po1b = ps_o.tile([HD + 1, 512], f32, name="po", tag="o")
                    attention_pair(qq, 1, po0b, po1b)
                    if qq > 0:
                        proj(qq - 1)
                    normalize_pair(qq, 1, po0b, po1b, ob23)
                proj(NQ - 1, tail=True)

    return nc


def _get_nc():
    if "nc" not in _cache:
        nc = _build_nc()
        nc.finalize()  # runs the Bacc pass pipeline (reg alloc, wait splitting)
        _cache["nc"] = nc
    return _cache["nc"]


def _make_in_maps(x, attn_mask, W_qkv, b_qkv, W_proj):
    x = np.asarray(x, dtype=np.float32)
    attn_mask = np.asarray(attn_mask)
    W_qkv = np.asarray(W_qkv, dtype=np.float32)
    b_qkv = np.asarray(b_qkv, dtype=np.float32)
    W_proj = np.asarray(W_proj, dtype=np.float32)

    xT = [np.ascontiguousarray(x[b].T.astype(BF16)) for b in range(2)]

    in_maps = []
    for core in range(8):
        b, g = core // 4, core % 4
        s = slice(CL * g, CL * (g + 1))
        wq = W_qkv[:, 0 * C:1 * C][:, s]
        wk = W_qkv[:, 1 * C:2 * C][:, s]
        wv = W_qkv[:, 2 * C:3 * C][:, s]
        bq = b_qkv[0 * C:1 * C][s]
        bk = b_qkv[1 * C:2 * C][s]
        bv = b_qkv[2 * C:3 * C][s]
        bqk = np.concatenate([bq, bk]).reshape(4, P).T  # [128,4], f = fi*128+p
        kbias = (attn_mask[b] != 0).astype(np.float32)  # 0/1 key mask
        in_maps.append({
            "xT": xT[b],
            "wqk": np.ascontiguousarray(
                np.concatenate([wq, wk], axis=1).astype(BF16)),
            "wv": np.ascontiguousarray(wv.astype(BF16)),
            "bqk": np.ascontiguousarray(bqk),
            "bv": np.ascontiguousarray(bv.reshape(1, CL)),
            "kbias": np.ascontiguousarray(kbias.reshape(16, P).T),
            "wproj": np.ascontiguousarray(W_proj[s, :].astype(BF16)),
        })
    return in_maps


def kernel(x, attn_mask, W_qkv, b_qkv, W_proj, b_proj, _trace=False):
    from concourse.bass_utils import run_bass_kernel_spmd

    nc = _get_nc()
    in_maps = _make_in_maps(x, attn_mask, W_qkv, b_qkv, W_proj)
    res = run_bass_kernel_spmd(nc, in_maps, list(range(8)), trace=_trace)
    outs = res.results

    b_proj = np.asarray(b_proj, dtype=np.float32)
    y = np.empty((2, T, C), dtype=np.float32)
    for b in range(2):
        acc = outs[b * 4]["yT"].T.astype(np.float32)
        for g in range(1, 4):
            acc = acc + outs[b * 4 + g]["yT"].T.astype(np.float32)
        y[b] = acc + b_proj
    if _trace:
        return y, res
    return y


# revision 34
# speedup vs baseline: 1.1646x; 1.0126x over previous
# Causal self-attention on 8 NeuronCores (Trainium2, Bass/Tile). v4.
#
# Problem: B=2, T=2048, C=1024, H=16 heads (hd=64).
#   qkv = x @ W_qkv + b_qkv ; per-head causal softmax attention ; y = att_out @ W_proj + b_proj
#
# Sharding: tensor-parallel over heads x data-parallel over batch.
#   core = b*4 + g   (b in {0,1} batch, g in {0..3} head group of 4 heads)
#
# v6 (from v3).  Changes driven by HW NTFF traces:
#   - all matmuls bf16 (was f32r): f32r disables fast-weight-load, so every
#     matmul serialized with its 128-cycle LDWEIGHTS (MATMUL busy 232us +
#     LDW 126us).  bf16 hides the weight load (FWL + background buffer):
#     MATMUL busy -> ~141us.  bf16 error (~0.5% rel) is well inside the 2e-2
#     gate.  fp8 was tried and FAILS (6.4e-2): attention output is an average
#     of uncorrelated v's, so element noise in scores/probs/values passes
#     through at full relative strength.
#   - S matmuls (contraction = hd = 64) head-pair packed: heads (2h, 2h+1)
#     live on partitions 0:64 / 64:128 of qkT, so their S matmuls emit as
#     concurrent 64x128 row tiles (T0/T8) into different PSUM banks -> the
#     K=64 half-array waste disappears.
#   - softmax reciprocal: DVE reciprocal on [1,512] is ~4us (single lane,
#     ~9.4 cyc/elem).  Widen both heads' denominator rows into [128,8] via
#     one DMA, one reciprocal (~200ns), one DMA back, one broadcast for both
#     heads.  63us -> ~2us.
#   - even head's normalize multiply writes AT[0:64] directly (lane-aligned);
#     only the odd head needs the partition-moving DMA.
#   - proj evacuation all on DVE (ACT is busy with exp during B2).
#   - x / W_qkv / W_v / W_proj shipped as bf16: halves the phase-A DMA.

import numpy as np
import ml_dtypes

T = 2048
C = 1024
HL = 4          # heads per core
HD = 64
CL = HL * HD    # 256 local channels
P = 128

BF16 = ml_dtypes.bfloat16
FP8 = ml_dtypes.float8_e4m3

_cache = {}


def _build_nc():
    import concourse.bass as bass
    import concourse.mybir as mybir
    import concourse.tile as tile
    from concourse import bacc
    from contextlib import ExitStack

    f32 = mybir.dt.float32
    bf16 = mybir.dt.bfloat16
    f8 = mybir.dt.float8e4
    DR = mybir.MatmulPerfMode.DoubleRow
    EXP = mybir.ActivationFunctionType.Exp
    IDENT = mybir.ActivationFunctionType.Identity

    nc = bacc.Bacc("TRN2", target_bir_lowering=False)
    xT_d = nc.declare_dram_parameter("xT", [C, T], bf16, isOutput=False)
    wqk_d = nc.declare_dram_parameter("wqk", [C, 2 * CL], bf16, isOutput=False)
    wv_d = nc.declare_dram_parameter("wv", [C, CL], bf16, isOutput=False)
    bqk_d = nc.declare_dram_parameter("bqk", [P, 4], f32, isOutput=False)
    bv_d = nc.declare_dram_parameter("bv", [1, CL], f32, isOutput=False)
    kbias_d = nc.declare_dram_parameter("kbias", [P, 16], f32, isOutput=False)
    wproj_d = nc.declare_dram_parameter("wproj", [CL, C], bf16, isOutput=False)
    yT_d = nc.declare_dram_parameter("yT", [C, T], bf16, isOutput=True)

    NT = T // P       # 16 t-tiles of 128
    NCC = C // P      # 8 contraction chunks of 128
    NQ = T // 512     # 4 q-quads of 512

    with tile.TileContext(nc) as tc, ExitStack() as ctx:
        singles = ctx.enter_context(tc.tile_pool(name="singles", bufs=1))

        # persistent SBUF
        qkT = singles.tile([P, 4, T], bf16)        # rows: [q f0,q f1,k f0,k f1]
        vv = singles.tile([P, NT, HL, HD + 1], bf16)  # v + ones col per head
        AT = singles.tile([P, 2, T], bf16)         # attention out^T (c' x t)
        tri01 = singles.tile([P, P], bf16)         # lower-tri 1.0 / 0.0
        kbias_sb = singles.tile([P, 16], f32)
        bqk_sb = singles.tile([P, 4], f32)
        bv_sb = singles.tile([P, HL, HD], f32)
        wproj_sb = singles.tile([P, 2, C], bf16)
        xc = singles.tile([P, NCC, T], bf16)
        wqkc = singles.tile([P, NCC, 2 * CL], bf16)
        wv_sb = singles.tile([P, NCC, CL], bf16)

        # weights/params on the gpsimd (SWDGE) queue, xT chunks on the sync
        # (HWDGE) queue.  The first q/k matmul needs only wqkc[:,0,:] +
        # xc[:,0,:], so those are issued FIRST on their queues; tri01 setup
        # (gpsimd engine work, needed only in B1) comes after the DMA issues.
        for ci in range(NCC):
            nc.sync.dma_start(out=xc[:, ci, :], in_=xT_d[ci * P:(ci + 1) * P, :])
        for ci in range(2):
            nc.gpsimd.dma_start(
                out=wqkc[:, ci, :], in_=wqk_d[ci * P:(ci + 1) * P, :]
            )
        nc.gpsimd.dma_start(out=kbias_sb, in_=kbias_d[:])
        nc.gpsimd.dma_start(out=bqk_sb, in_=bqk_d[:])
        nc.gpsimd.dma_start(
            out=bv_sb,
            in_=bv_d[:].rearrange("o (h d) -> o h d", h=HL).to_broadcast([P, HL, HD]),
        )
        for ci in range(2, NCC):
            nc.gpsimd.dma_start(
                out=wqkc[:, ci, :], in_=wqk_d[ci * P:(ci + 1) * P, :]
            )
        nc.gpsimd.dma_start(out=wv_sb, in_=wv_d[:].rearrange("(o p) n -> p o n", p=P))
        nc.gpsimd.dma_start(out=wproj_sb, in_=wproj_d[:].rearrange("(o p) n -> p o n", p=P))
        nc.vector.memset(vv[:, :, :, HD], 1.0)

        # tri01[k, q] = 1.0 where q >= k else 0.0
        nc.gpsimd.memset(tri01, 1.0)
        nc.gpsimd.affine_select(
            out=tri01,
            in_=tri01,
            compare_op=mybir.AluOpType.is_ge,
            fill=0.0,
            base=0,
            pattern=[[1, P]],
            channel_multiplier=-1,
        )

        with (
            tc.tile_pool(name="ptp", bufs=4) as ptp,
            tc.tile_pool(name="ep", bufs=3) as ep,
            tc.tile_pool(name="yst", bufs=3) as yst,
        ):
            phA_cm = tc.tile_pool(name="phA", bufs=1, space="PSUM")
            phA = phA_cm.__enter__()
            # ---- q/k group A (fi 0,2 = q,k of heads 0/1): ci-outer ----
            groupA = [(fi, tj) for fi in (0, 2) for tj in range(4)]
            pqA = [
                phA.tile([P, 512], f32, name=f"pqA{gi}")
                for gi in range(len(groupA))
            ]
            for ci in range(NCC):
                for gi, (fi, tj) in enumerate(groupA):
                    nc.tensor.matmul(
                        pqA[gi],
                        lhsT=wqkc[:, ci, fi * P:(fi + 1) * P],
                        rhs=xc[:, ci, tj * 512:(tj + 1) * 512],
                        start=(ci == 0),
                        stop=(ci == NCC - 1),
                        skip_group_check=True,
                    )
            for gi, (fi, tj) in enumerate(groupA):
                # bias-add evacuation on the (idle in phase A) ACT engine
                nc.scalar.activation(
                    out=qkT[:, fi, tj * 512:(tj + 1) * 512],
                    in_=pqA[gi],
                    func=IDENT,
                    bias=bqk_sb[:, fi:fi + 1],
                )

            # ---- v pass: 16 half-bank accum groups reusing the 8 banks ----
            for ti in range(NT):
                pv = pqA[ti % 8][:, (ti // 8) * CL:(ti // 8 + 1) * CL]
                for ci in range(NCC):
                    nc.tensor.matmul(
                        pv,
                        lhsT=xc[:, ci, ti * P:(ti + 1) * P],
                        rhs=wv_sb[:, ci, :],
                        start=(ci == 0),
                        stop=(ci == NCC - 1),
                        skip_group_check=True,
                    )
                nc.vector.tensor_add(
                    out=vv[:, ti, :, 0:HD],
                    in0=pv.rearrange("p (h d) -> p h d", h=HL),
                    in1=bv_sb,
                )
                # key-padding mask: zero this key's v row AND its ones-col
                nc.vector.tensor_scalar_mul(
                    out=vv[:, ti, :, :],
                    in0=vv[:, ti, :, :],
                    scalar1=kbias_sb[:, ti:ti + 1],
                )

            phA_cm.__exit__(None, None, None)

            groupB = [(fi, tj) for fi in (1, 3) for tj in range(4)]

            def qk_groupB(k):
                fi, tj = groupB[k]
                pq = ps_y.tile([P, 512], f32, name=f"pqB", tag="y")
                for ci in range(NCC):
                    nc.tensor.matmul(
                        pq,
                        lhsT=wqkc[:, ci, fi * P:(fi + 1) * P],
                        rhs=xc[:, ci, tj * 512:(tj + 1) * 512],
                        start=(ci == 0),
                        stop=(ci == NCC - 1),
                    )
                nc.vector.tensor_scalar_add(
                    out=qkT[:, fi, tj * 512:(tj + 1) * 512],
                    in0=pq,
                    scalar1=bqk_sb[:, fi:fi + 1],
                )

            def attention_pair(qq, hp, po0, po1):
                # heads (2*hp, 2*hp+1) processed together.  Their q/k rows sit
                # on SBUF partitions 0:64 / 64:128 of qkT, so the two S
                # matmuls of a chunk are emitted back-to-back as 64x128 row
                # tiles (T0 / T8) writing different PSUM banks -> they run
                # CONCURRENTLY on the PE array (2x on the K=64 S matmuls).
                qs = qq * 512
                fo = hp
                qT = qkT[:, fo, :]
                kT = qkT[:, 2 + fo, :]
                pos = (po0, po1)
                # full (below-diagonal) chunks, in groups of 2 to keep the
                # 64-row-mode <-> 128-row-mode switches as rare as v4
                for jp in range(2 * qq):
                    j0 = 2 * jp
                    pss = [
                        ps_s.tile([P, 2, 512], f32, name="ps2", tag="s")
                        for _ in range(2)
                    ]
                    pTs = [
                        ptp.tile([P, 2, 512], bf16, name="pT2", tag="p")
                        for _ in range(2)
                    ]
                    for m in range(2):
                        for i in range(2):
                            b = i * HD
                            nc.tensor.matmul(
                                pss[m][:, i, :],
                                lhsT=kT[b:b + HD, (j0 + m) * P:(j0 + m + 1) * P],
                                rhs=qT[b:b + HD, qs:qs + 512],
                                start=True,
                                stop=True,
                            )
                    for m in range(2):
                        nc.scalar.activation(
                            out=pTs[m], in_=pss[m], func=EXP, scale=0.125,
                        )
                    for m in range(2):
                        for i in range(2):
                            nc.tensor.matmul(
                                pos[i],
                                lhsT=vv[:, j0 + m, 2 * hp + i, :],
                                rhs=pTs[m][:, i, :],
                                start=(j0 + m == 0),
                                stop=False,
                            )
                # diagonal-region chunks (o = 0..3), width-trimmed to >=256
                for o in range(4):
                    j = 4 * qq + o
                    d0 = 128 * o
                    a = min(d0, 256)
                    ps = ps_s.tile([P, 2, 512], f32, name="psd", tag="s")
                    pT = ptp.tile([P, 2, 512], bf16, name="pTd", tag="p")
                    for i in range(2):
                        b = i * HD
                        nc.tensor.matmul(
                            ps[:, i, a:],
                            lhsT=kT[b:b + HD, j * P:(j + 1) * P],
                            rhs=qT[b:b + HD, qs + a:qs + 512],
                            start=True,
                            stop=True,
                        )
                    nc.scalar.activation(
                        out=pT[:, :, d0:], in_=ps[:, :, d0:],
                        func=EXP, scale=0.125,
                    )
                    for i in range(2):
                        nc.vector.tensor_mul(
                            out=pT[:, i, d0:d0 + P],
                            in0=pT[:, i, d0:d0 + P],
                            in1=tri01,
                        )
                        if d0 > a:
                            # o==3: cols [256,384) are causally invalid
                            nc.vector.memset(pT[:, i, a:d0], 0.0)
                    for i in range(2):
                        nc.tensor.matmul(
                            pos[i][:, a:],
                            lhsT=vv[:, j, 2 * hp + i, :],
                            rhs=pT[:, i, a:],
                            start=(j == 0),
                            stop=(o == 3),
                        )

            def normalize_pair(qq, hp, po0, po1, ob2):
                # denominators of heads (2*hp, 2*hp+1) sit on partition 64 of
                # po0/po1.  Widen [1024] -> [128,8] in ONE DMA so the
                # reciprocal runs 8 elem/lane instead of 512 on one lane;
                # scatter back in one DMA; one broadcast for both heads.
                # The two row-copies run in parallel on ACT and DVE - this
                # chain's latency gates the proj filler each iteration.
                qs = qq * 512
                # Evacuate BOTH accumulators to SBUF right away (full-tile
                # copies, ACT + DVE in parallel) so the po PSUM banks free up
                # at the START of this chain, not after the normalize muls -
                # otherwise the next quad's AV matmuls stall ~5us on the
                # ps_o pool.  Row 64 is the ones-column denominator.
                st = ep.tile([HD + 1, 2, 512], f32, name="st", tag="st")
                nc.scalar.activation(out=st[:, 0, :], in_=po0, func=IDENT)
                nc.vector.tensor_copy(out=st[:, 1, :], in_=po1)
                dwide = ep.tile([P, 8], f32, name="dwide", tag="dw")
                nc.sync.dma_start(
                    out=dwide,
                    in_=st[HD:HD + 1, :, :].rearrange(
                        "o h (p c) -> o (h p) c", p=P // 2
                    ),
                )
                rwide = ep.tile([P, 8], f32, name="rwide", tag="rw")
                with nc.allow_low_precision(
                    reason="reciprocal of softmax denom; 2^-11 rel"
                ):
                    nc.vector.reciprocal(out=rwide, in_=dwide)
                den2 = ep.tile([1, 1024], f32, name="den2", tag="den")
                nc.sync.dma_start(
                    out=den2.rearrange("o (p c) -> o p c", p=P),
                    in_=rwide,
                )
                rb = ep.tile([HD, 1024], f32, name="rb", tag="rb")
                nc.gpsimd.partition_broadcast(rb, den2)
                # even head (partitions 0:64 of AT): the normalize multiply
                # writes AT directly - no partition-moving DMA needed
                nc.vector.tensor_mul(
                    out=AT[0:HD, hp, qs:qs + 512], in0=st[0:HD, 0, :],
                    in1=rb[:, 0:512],
                )
                # odd head: DVE is lane-aligned, so route via ob2 + DMA to
                # reach AT partitions 64:128
                nc.vector.tensor_mul(
                    out=ob2[:, 1, :], in0=st[0:HD, 1, :],
                    in1=rb[:, 512:1024],
                )
                nc.gpsimd.dma_start(
                    out=AT[HD:P, hp, qs:qs + 512],
                    in_=ob2[:, 1, :],
                )

            def proj(qq, tail=False):
                for co in range(C // P):
                    py = ps_y.tile([P, 512], f32, name="py", tag="y")
                    for cc in range(2):
                        nc.tensor.matmul(
                            py,
                            lhsT=wproj_sb[:, cc, co * P:(co + 1) * P],
                            rhs=AT[:, cc, qq * 512:(qq + 1) * 512],
                            start=(cc == 0),
                            stop=(cc == 1),
                        )
                    yt = yst.tile([P, 512], bf16, name="yt", tag="yt")
                    if tail and co % 2 == 0:
                        # final quad: ACT is idle at the tail - split the
                        # evacuation so it doesn't serialize on DVE
                        nc.scalar.copy(out=yt, in_=py)
                    else:
                        nc.vector.tensor_copy(out=yt, in_=py)
                    dma_eng = nc.sync if co % 2 == 0 else nc.gpsimd
                    dma_eng.dma_start(
                        out=yT_d[co * P:(co + 1) * P, qq * 512:(qq + 1) * 512],
                        in_=yt,
                    )

            with (
                tc.tile_pool(name="ps_s", bufs=2, space="PSUM") as ps_s,
                tc.tile_pool(name="ps_o", bufs=2, space="PSUM") as ps_o,
                tc.tile_pool(name="ps_y", bufs=2, space="PSUM") as ps_y,
            ):
                # ---- B: single loop, head pairs interleaved per quad.
                # Each pair's normalize chain overlaps the other pair's
                # attention / fillers.  qk_groupB(qq) + qk_groupB(4+qq)
                # produce q (f1, quad qq) and k (f3, chunk range tj=qq) of
                # heads 2/3 just-in-time: attention_pair(qq, 1) needs k
                # columns 0:(qq+1)*512, i.e. tj <= qq - all emitted by then.
                for qq in range(NQ):
                    ob01 = ep.tile([HD, 2, 512], bf16, name="ob01", tag="ob2")
                    po0 = ps_o.tile([HD + 1, 512], f32, name="po", tag="o")
                    po1 = ps_o.tile([HD + 1, 512], f32, name="po", tag="o")
                    attention_pair(qq, 0, po0, po1)
                    qk_groupB(qq)
                    qk_groupB(4 + qq)
                    normalize_pair(qq, 0, po0, po1, ob01)
                    ob23 = ep.tile([HD, 2, 512], bf16, name="ob23", tag="ob2")
                    po0b = ps_o.tile([HD + 1, 512], f32, name="po", tag="o")
                    po1b = ps_o.tile([HD + 1, 512], f32, name="po", tag="o")
                    attention_pair(qq, 1, po0b, po1b)
                    if qq > 0:
                        proj(qq - 1)
                    normalize_pair(qq, 1, po0b, po1b, ob23)
                proj(NQ - 1, tail=True)

    return nc


def _get_nc():
    if "nc" not in _cache:
        nc = _build_nc()
        nc.finalize()  # runs the Bacc pass pipeline (reg alloc, wait splitting)
        _cache["nc"] = nc
    return _cache["nc"]


def _make_in_maps(x, attn_mask, W_qkv, b_qkv, W_proj):
    x = np.asarray(x, dtype=np.float32)
    attn_mask = np.asarray(attn_mask)
    W_qkv = np.asarray(W_qkv, dtype=np.float32)
    b_qkv = np.asarray(b_qkv, dtype=np.float32)
    W_proj = np.asarray(W_proj, dtype=np.float32)

    xT = [np.ascontiguousarray(x[b].T.astype(BF16)) for b in range(2)]

    in_maps = []
    for core in range(8):
        b, g = core // 4, core % 4
        s = slice(CL * g, CL * (g + 1))
        wq = W_qkv[:, 0 * C:1 * C][:, s]
        wk = W_qkv[:, 1 * C:2 * C][:, s]
        wv = W_qkv[:, 2 * C:3 * C][:, s]
        bq = b_qkv[0 * C:1 * C][s]
        bk = b_qkv[1 * C:2 * C][s]
        bv = b_qkv[2 * C:3 * C][s]
        bqk = np.concatenate([bq, bk]).reshape(4, P).T  # [128,4], f = fi*128+p
        kbias = (attn_mask[b] != 0).astype(np.float32)  # 0/1 key mask
        in_maps.append({
            "xT": xT[b],
            "wqk": np.ascontiguousarray(
                np.concatenate([wq, wk], axis=1).astype(BF16)),
            "wv": np.ascontiguousarray(wv.astype(BF16)),
            "bqk": np.ascontiguousarray(bqk),
            "bv": np.ascontiguousarray(bv.reshape(1, CL)),
            "kbias": np.ascontiguousarray(kbias.reshape(16, P).T),
            "wproj": np.ascontiguousarray(W_proj[s, :].astype(BF16)),
        })
    return in_maps


def kernel(x, attn_mask, W_qkv, b_qkv, W_proj, b_proj, _trace=False):
    from concourse.bass_utils import run_bass_kernel_spmd

    nc = _get_nc()
    in_maps = _make_in_maps(x, attn_mask, W_qkv, b_qkv, W_proj)
    res = run_bass_kernel_spmd(nc, in_maps, list(range(8)), trace=_trace)
    outs = res.results

    b_proj = np.asarray(b_proj, dtype=np.float32)
    y = np.empty((2, T, C), dtype=np.float32)
    for b in range(2):
        acc = outs[b * 4]["yT"].T.astype(np.float32)
        for g in range(1, 4):
            acc = acc + outs[b * 4 + g]["yT"].T.astype(np.float32)
        y[b] = acc + b_proj
    if _trace:
        return y, res
    return y


# revision 35
# speedup vs baseline: 1.1672x; 1.0022x over previous
# Causal self-attention on 8 NeuronCores (Trainium2, Bass/Tile). v4.
#
# Problem: B=2, T=2048, C=1024, H=16 heads (hd=64).
#   qkv = x @ W_qkv + b_qkv ; per-head causal softmax attention ; y = att_out @ W_proj + b_proj
#
# Sharding: tensor-parallel over heads x data-parallel over batch.
#   core = b*4 + g   (b in {0,1} batch, g in {0..3} head group of 4 heads)
#
# v6 (from v3).  Changes driven by HW NTFF traces:
#   - all matmuls bf16 (was f32r): f32r disables fast-weight-load, so every
#     matmul serialized with its 128-cycle LDWEIGHTS (MATMUL busy 232us +
#     LDW 126us).  bf16 hides the weight load (FWL + background buffer):
#     MATMUL busy -> ~141us.  bf16 error (~0.5% rel) is well inside the 2e-2
#     gate.  fp8 was tried and FAILS (6.4e-2): attention output is an average
#     of uncorrelated v's, so element noise in scores/probs/values passes
#     through at full relative strength.
#   - S matmuls (contraction = hd = 64) head-pair packed: heads (2h, 2h+1)
#     live on partitions 0:64 / 64:128 of qkT, so their S matmuls emit as
#     concurrent 64x128 row tiles (T0/T8) into different PSUM banks -> the
#     K=64 half-array waste disappears.
#   - softmax reciprocal: DVE reciprocal on [1,512] is ~4us (single lane,
#     ~9.4 cyc/elem).  Widen both heads' denominator rows into [128,8] via
#     one DMA, one reciprocal (~200ns), one DMA back, one broadcast for both
#     heads.  63us -> ~2us.
#   - even head's normalize multiply writes AT[0:64] directly (lane-aligned);
#     only the odd head needs the partition-moving DMA.
#   - proj evacuation all on DVE (ACT is busy with exp during B2).
#   - x / W_qkv / W_v / W_proj shipped as bf16: halves the phase-A DMA.

import numpy as np
import ml_dtypes

T = 2048
C = 1024
HL = 4          # heads per core
HD = 64
CL = HL * HD    # 256 local channels
P = 128

BF16 = ml_dtypes.bfloat16
FP8 = ml_dtypes.float8_e4m3

_cache = {}


def _build_nc():
    import concourse.bass as bass
    import concourse.mybir as mybir
    import concourse.tile as tile
    from concourse import bacc
    from contextlib import ExitStack

    f32 = mybir.dt.float32
    bf16 = mybir.dt.bfloat16
    f8 = mybir.dt.float8e4
    DR = mybir.MatmulPerfMode.DoubleRow
    EXP = mybir.ActivationFunctionType.Exp
    IDENT = mybir.ActivationFunctionType.Identity

    nc = bacc.Bacc("TRN2", target_bir_lowering=False)
    xT_d = nc.declare_dram_parameter("xT", [C, T], bf16, isOutput=False)
    wqk_d = nc.declare_dram_parameter("wqk", [C, 2 * CL], bf16, isOutput=False)
    wv_d = nc.declare_dram_parameter("wv", [C, CL], bf16, isOutput=False)
    bqk_d = nc.declare_dram_parameter("bqk", [P, 4], f32, isOutput=False)
    bv_d = nc.declare_dram_parameter("bv", [1, CL], f32, isOutput=False)
    kbias_d = nc.declare_dram_parameter("kbias", [P, 16], f32, isOutput=False)
    wproj_d = nc.declare_dram_parameter("wproj", [CL, C], bf16, isOutput=False)
    yT_d = nc.declare_dram_parameter("yT", [C, T], bf16, isOutput=True)

    NT = T // P       # 16 t-tiles of 128
    NCC = C // P      # 8 contraction chunks of 128
    NQ = T // 512     # 4 q-quads of 512

    with tile.TileContext(nc) as tc, ExitStack() as ctx:
        singles = ctx.enter_context(tc.tile_pool(name="singles", bufs=1))

        # persistent SBUF
        qkT = singles.tile([P, 4, T], bf16)        # rows: [q f0,q f1,k f0,k f1]
        vv = singles.tile([P, NT, HL, HD + 1], bf16)  # v + ones col per head
        AT = singles.tile([P, 2, T], bf16)         # attention out^T (c' x t)
        tri01 = singles.tile([P, P], bf16)         # lower-tri 1.0 / 0.0
        kbias_sb = singles.tile([P, 16], f32)
        bqk_sb = singles.tile([P, 4], f32)
        bv_sb = singles.tile([P, HL, HD], f32)
        wproj_sb = singles.tile([P, 2, C], bf16)
        xc = singles.tile([P, NCC, T], bf16)
        wqkc = singles.tile([P, NCC, 2 * CL], bf16)
        wv_sb = singles.tile([P, NCC, CL], bf16)

        # weights/params on the gpsimd (SWDGE) queue, xT chunks on the sync
        # (HWDGE) queue.  The first q/k matmul needs only wqkc[:,0,:] +
        # xc[:,0,:], so those are issued FIRST on their queues; tri01 setup
        # (gpsimd engine work, needed only in B1) comes after the DMA issues.
        # chunk 0 split so the very first matmul (fi=0, tj=0) starts after
        # 128KB of x + 32KB of weights instead of the full 640KB
        nc.sync.dma_start(out=xc[:, 0, 0:512], in_=xT_d[0:P, 0:512])
        nc.sync.dma_start(out=xc[:, 0, 512:], in_=xT_d[0:P, 512:])
        for ci in range(1, NCC):
            nc.sync.dma_start(out=xc[:, ci, :], in_=xT_d[ci * P:(ci + 1) * P, :])
        nc.gpsimd.dma_start(out=wqkc[:, 0, 0:P], in_=wqk_d[0:P, 0:P])
        nc.gpsimd.dma_start(out=wqkc[:, 0, P:], in_=wqk_d[0:P, P:])
        nc.gpsimd.dma_start(out=wqkc[:, 1, :], in_=wqk_d[P:2 * P, :])
        nc.gpsimd.dma_start(out=kbias_sb, in_=kbias_d[:])
        nc.gpsimd.dma_start(out=bqk_sb, in_=bqk_d[:])
        nc.gpsimd.dma_start(
            out=bv_sb,
            in_=bv_d[:].rearrange("o (h d) -> o h d", h=HL).to_broadcast([P, HL, HD]),
        )
        for ci in range(2, NCC):
            nc.gpsimd.dma_start(
                out=wqkc[:, ci, :], in_=wqk_d[ci * P:(ci + 1) * P, :]
            )
        nc.gpsimd.dma_start(out=wv_sb, in_=wv_d[:].rearrange("(o p) n -> p o n", p=P))
        nc.gpsimd.dma_start(out=wproj_sb, in_=wproj_d[:].rearrange("(o p) n -> p o n", p=P))
        nc.vector.memset(vv[:, :, :, HD], 1.0)

        # tri01[k, q] = 1.0 where q >= k else 0.0
        nc.gpsimd.memset(tri01, 1.0)
        nc.gpsimd.affine_select(
            out=tri01,
            in_=tri01,
            compare_op=mybir.AluOpType.is_ge,
            fill=0.0,
            base=0,
            pattern=[[1, P]],
            channel_multiplier=-1,
        )

        with (
            tc.tile_pool(name="ptp", bufs=4) as ptp,
            tc.tile_pool(name="ep", bufs=3) as ep,
            tc.tile_pool(name="yst", bufs=3) as yst,
        ):
            phA_cm = tc.tile_pool(name="phA", bufs=1, space="PSUM")
            phA = phA_cm.__enter__()
            # ---- q/k group A (fi 0,2 = q,k of heads 0/1): ci-outer ----
            groupA = [(fi, tj) for fi in (0, 2) for tj in range(4)]
            pqA = [
                phA.tile([P, 512], f32, name=f"pqA{gi}")
                for gi in range(len(groupA))
            ]
            for ci in range(NCC):
                for gi, (fi, tj) in enumerate(groupA):
                    nc.tensor.matmul(
                        pqA[gi],
                        lhsT=wqkc[:, ci, fi * P:(fi + 1) * P],
                        rhs=xc[:, ci, tj * 512:(tj + 1) * 512],
                        start=(ci == 0),
                        stop=(ci == NCC - 1),
                        skip_group_check=True,
                    )
            for gi, (fi, tj) in enumerate(groupA):
                # bias-add evacuation on the (idle in phase A) ACT engine
                nc.scalar.activation(
                    out=qkT[:, fi, tj * 512:(tj + 1) * 512],
                    in_=pqA[gi],
                    func=IDENT,
                    bias=bqk_sb[:, fi:fi + 1],
                )

            # ---- v pass: 16 half-bank accum groups reusing the 8 banks ----
            for ti in range(NT):
                pv = pqA[ti % 8][:, (ti // 8) * CL:(ti // 8 + 1) * CL]
                for ci in range(NCC):
                    nc.tensor.matmul(
                        pv,
                        lhsT=xc[:, ci, ti * P:(ti + 1) * P],
                        rhs=wv_sb[:, ci, :],
                        start=(ci == 0),
                        stop=(ci == NCC - 1),
                        skip_group_check=True,
                    )
                nc.vector.tensor_add(
                    out=vv[:, ti, :, 0:HD],
                    in0=pv.rearrange("p (h d) -> p h d", h=HL),
                    in1=bv_sb,
                )
                # key-padding mask: zero this key's v row AND its ones-col
                nc.vector.tensor_scalar_mul(
                    out=vv[:, ti, :, :],
                    in0=vv[:, ti, :, :],
                    scalar1=kbias_sb[:, ti:ti + 1],
                )

            phA_cm.__exit__(None, None, None)

            groupB = [(fi, tj) for fi in (1, 3) for tj in range(4)]

            def qk_groupB(k):
                fi, tj = groupB[k]
                pq = ps_y.tile([P, 512], f32, name=f"pqB", tag="y")
                for ci in range(NCC):
                    nc.tensor.matmul(
                        pq,
                        lhsT=wqkc[:, ci, fi * P:(fi + 1) * P],
                        rhs=xc[:, ci, tj * 512:(tj + 1) * 512],
                        start=(ci == 0),
                        stop=(ci == NCC - 1),
                    )
                nc.vector.tensor_scalar_add(
                    out=qkT[:, fi, tj * 512:(tj + 1) * 512],
                    in0=pq,
                    scalar1=bqk_sb[:, fi:fi + 1],
                )

            def attention_pair(qq, hp, po0, po1):
                # heads (2*hp, 2*hp+1) processed together.  Their q/k rows sit
                # on SBUF partitions 0:64 / 64:128 of qkT, so the two S
                # matmuls of a chunk are emitted back-to-back as 64x128 row
                # tiles (T0 / T8) writing different PSUM banks -> they run
                # CONCURRENTLY on the PE array (2x on the K=64 S matmuls).
                qs = qq * 512
                fo = hp
                qT = qkT[:, fo, :]
                kT = qkT[:, 2 + fo, :]
                pos = (po0, po1)
                # full (below-diagonal) chunks, in groups of 2 to keep the
                # 64-row-mode <-> 128-row-mode switches as rare as v4
                for jp in range(2 * qq):
                    j0 = 2 * jp
                    pss = [
                        ps_s.tile([P, 2, 512], f32, name="ps2", tag="s")
                        for _ in range(2)
                    ]
                    pTs = [
                        ptp.tile([P, 2, 512], bf16, name="pT2", tag="p")
                        for _ in range(2)
                    ]
                    for m in range(2):
                        for i in range(2):
                            b = i * HD
                            nc.tensor.matmul(
                                pss[m][:, i, :],
                                lhsT=kT[b:b + HD, (j0 + m) * P:(j0 + m + 1) * P],
                                rhs=qT[b:b + HD, qs:qs + 512],
                                start=True,
                                stop=True,
                            )
                    for m in range(2):
                        nc.scalar.activation(
                            out=pTs[m], in_=pss[m], func=EXP, scale=0.125,
                        )
                    for m in range(2):
                        for i in range(2):
                            nc.tensor.matmul(
                                pos[i],
                                lhsT=vv[:, j0 + m, 2 * hp + i, :],
                                rhs=pTs[m][:, i, :],
                                start=(j0 + m == 0),
                                stop=False,
                            )
                # diagonal-region chunks (o = 0..3), width-trimmed to >=256
                for o in range(4):
                    j = 4 * qq + o
                    d0 = 128 * o
                    a = min(d0, 256)
                    ps = ps_s.tile([P, 2, 512], f32, name="psd", tag="s")
                    pT = ptp.tile([P, 2, 512], bf16, name="pTd", tag="p")
                    for i in range(2):
                        b = i * HD
                        nc.tensor.matmul(
                            ps[:, i, a:],
                            lhsT=kT[b:b + HD, j * P:(j + 1) * P],
                            rhs=qT[b:b + HD, qs + a:qs + 512],
                            start=True,
                            stop=True,
                        )
                    nc.scalar.activation(
                        out=pT[:, :, d0:], in_=ps[:, :, d0:],
                        func=EXP, scale=0.125,
                    )
                    for i in range(2):
                        nc.vector.tensor_mul(
                            out=pT[:, i, d0:d0 + P],
                            in0=pT[:, i, d0:d0 + P],
                            in1=tri01,
                        )
                        if d0 > a:
                            # o==3: cols [256,384) are causally invalid
                            nc.vector.memset(pT[:, i, a:d0], 0.0)
                    for i in range(2):
                        nc.tensor.matmul(
                            pos[i][:, a:],
                            lhsT=vv[:, j, 2 * hp + i, :],
                            rhs=pT[:, i, a:],
                            start=(j == 0),
                            stop=(o == 3),
                        )

            def normalize_pair(qq, hp, po0, po1, ob2):
                # denominators of heads (2*hp, 2*hp+1) sit on partition 64 of
                # po0/po1.  Widen [1024] -> [128,8] in ONE DMA so the
                # reciprocal runs 8 elem/lane instead of 512 on one lane;
                # scatter back in one DMA; one broadcast for both heads.
                # The two row-copies run in parallel on ACT and DVE - this
                # chain's latency gates the proj filler each iteration.
                qs = qq * 512
                # Evacuate BOTH accumulators to SBUF right away (full-tile
                # copies, ACT + DVE in parallel) so the po PSUM banks free up
                # at the START of this chain, not after the normalize muls -
                # otherwise the next quad's AV matmuls stall ~5us on the
                # ps_o pool.  Row 64 is the ones-column denominator.
                st = ep.tile([HD + 1, 2, 512], f32, name="st", tag="st")
                nc.scalar.activation(out=st[:, 0, :], in_=po0, func=IDENT)
                nc.vector.tensor_copy(out=st[:, 1, :], in_=po1)
                dwide = ep.tile([P, 8], f32, name="dwide", tag="dw")
                nc.sync.dma_start(
                    out=dwide,
                    in_=st[HD:HD + 1, :, :].rearrange(
                        "o h (p c) -> o (h p) c", p=P // 2
                    ),
                )
                rwide = ep.tile([P, 8], f32, name="rwide", tag="rw")
                with nc.allow_low_precision(
                    reason="reciprocal of softmax denom; 2^-11 rel"
                ):
                    nc.vector.reciprocal(out=rwide, in_=dwide)
                den2 = ep.tile([1, 1024], f32, name="den2", tag="den")
                nc.sync.dma_start(
                    out=den2.rearrange("o (p c) -> o p c", p=P),
                    in_=rwide,
                )
                rb = ep.tile([HD, 1024], f32, name="rb", tag="rb")
                nc.gpsimd.partition_broadcast(rb, den2)
                # even head (partitions 0:64 of AT): the normalize multiply
                # writes AT directly - no partition-moving DMA needed
                nc.vector.tensor_mul(
                    out=AT[0:HD, hp, qs:qs + 512], in0=st[0:HD, 0, :],
                    in1=rb[:, 0:512],
                )
                # odd head: DVE is lane-aligned, so route via ob2 + DMA to
                # reach AT partitions 64:128
                nc.vector.tensor_mul(
                    out=ob2[:, 1, :], in0=st[0:HD, 1, :],
                    in1=rb[:, 512:1024],
                )
                nc.gpsimd.dma_start(
                    out=AT[HD:P, hp, qs:qs + 512],
                    in_=ob2[:, 1, :],
                )

            def proj(qq, tail=False):
                for co in range(C // P):
                    py = ps_y.tile([P, 512], f32, name="py", tag="y")
                    for cc in range(2):
                        nc.tensor.matmul(
                            py,
                            lhsT=wproj_sb[:, cc, co * P:(co + 1) * P],
                            rhs=AT[:, cc, qq * 512:(qq + 1) * 512],
                            start=(cc == 0),
                            stop=(cc == 1),
                        )
                    yt = yst.tile([P, 512], bf16, name="yt", tag="yt")
                    if tail and co % 2 == 0:
                        # final quad: ACT is idle at the tail - split the
                        # evacuation so it doesn't serialize on DVE
                        nc.scalar.copy(out=yt, in_=py)
                    else:
                        nc.vector.tensor_copy(out=yt, in_=py)
                    dma_eng = nc.sync if co % 2 == 0 else nc.gpsimd
                    dma_eng.dma_start(
                        out=yT_d[co * P:(co + 1) * P, qq * 512:(qq + 1) * 512],
                        in_=yt,
                    )

            with (
                tc.tile_pool(name="ps_s", bufs=2, space="PSUM") as ps_s,
                tc.tile_pool(name="ps_o", bufs=2, space="PSUM") as ps_o,
                tc.tile_pool(name="ps_y", bufs=2, space="PSUM") as ps_y,
            ):
                # ---- B: single loop, head pairs interleaved per quad.
                # Each pair's normalize chain overlaps the other pair's
                # attention / fillers.  qk_groupB(qq) + qk_groupB(4+qq)
                # produce q (f1, quad qq) and k (f3, chunk range tj=qq) of
                # heads 2/3 just-in-time: attention_pair(qq, 1) needs k
                # columns 0:(qq+1)*512, i.e. tj <= qq - all emitted by then.
                for qq in range(NQ):
                    ob01 = ep.tile([HD, 2, 512], bf16, name="ob01", tag="ob2")
                    po0 = ps_o.tile([HD + 1, 512], f32, name="po", tag="o")
                    po1 = ps_o.tile([HD + 1, 512], f32, name="po", tag="o")
                    attention_pair(qq, 0, po0, po1)
                    qk_groupB(qq)
                    qk_groupB(4 + qq)
                    normalize_pair(qq, 0, po0, po1, ob01)
                    ob23 = ep.tile([HD, 2, 512], bf16, name="ob23", tag="ob2")
                    po0b = ps_o.tile([HD + 1, 512], f32, name="po", tag="o")
                    po1b = ps_o.tile([HD + 1, 512], f32, name="po", tag="o")
                    attention_pair(qq, 1, po0b, po1b)
                    if qq > 0:
                        proj(qq - 1)
                    normalize_pair(qq, 1, po0b, po1b, ob23)
                proj(NQ - 1, tail=True)

    return nc


def _get_nc():
    if "nc" not in _cache:
        nc = _build_nc()
        nc.finalize()  # runs the Bacc pass pipeline (reg alloc, wait splitting)
        _cache["nc"] = nc
    return _cache["nc"]


def _make_in_maps(x, attn_mask, W_qkv, b_qkv, W_proj):
    x = np.asarray(x, dtype=np.float32)
    attn_mask = np.asarray(attn_mask)
    W_qkv = np.asarray(W_qkv, dtype=np.float32)
    b_qkv = np.asarray(b_qkv, dtype=np.float32)
    W_proj = np.asarray(W_proj, dtype=np.float32)

    xT = [np.ascontiguousarray(x[b].T.astype(BF16)) for b in range(2)]

    in_maps = []
    for core in range(8):
        b, g = core // 4, core % 4
        s = slice(CL * g, CL * (g + 1))
        wq = W_qkv[:, 0 * C:1 * C][:, s]
        wk = W_qkv[:, 1 * C:2 * C][:, s]
        wv = W_qkv[:, 2 * C:3 * C][:, s]
        bq = b_qkv[0 * C:1 * C][s]
        bk = b_qkv[1 * C:2 * C][s]
        bv = b_qkv[2 * C:3 * C][s]
        bqk = np.concatenate([bq, bk]).reshape(4, P).T  # [128,4], f = fi*128+p
        kbias = (attn_mask[b] != 0).astype(np.float32)  # 0/1 key mask
        in_maps.append({
            "xT": xT[b],
            "wqk": np.ascontiguousarray(
                np.concatenate([wq, wk], axis=1).astype(BF16)),
            "wv": np.ascontiguousarray(wv.astype(BF16)),
            "bqk": np.ascontiguousarray(bqk),
            "bv": np.ascontiguousarray(bv.reshape(1, CL)),
            "kbias": np.ascontiguousarray(kbias.reshape(16, P).T),
            "wproj": np.ascontiguousarray(W_proj[s, :].astype(BF16)),
        })
    return in_maps


def kernel(x, attn_mask, W_qkv, b_qkv, W_proj, b_proj, _trace=False):
    from concourse.bass_utils import run_bass_kernel_spmd

    nc = _get_nc()
    in_maps = _make_in_maps(x, attn_mask, W_qkv, b_qkv, W_proj)
    res = run_bass_kernel_spmd(nc, in_maps, list(range(8)), trace=_trace)
    outs = res.results

    b_proj = np.asarray(b_proj, dtype=np.float32)
    y = np.empty((2, T, C), dtype=np.float32)
    for b in range(2):
        acc = outs[b * 4]["yT"].T.astype(np.float32)
        for g in range(1, 4):
            acc = acc + outs[b * 4 + g]["yT"].T.astype(np.float32)
        y[b] = acc + b_proj
    if _trace:
        return y, res
    return y
